# revision 12
# baseline (speedup 1.0000x reference)
"""Trainium2 Bass kernel for an LSTM poetry model.

Model: x = emb[input]; xg = x @ W_ih + b; LSTM scan over 128 steps;
logits = hidden @ lin_W.T + lin_b.

8-core SPMD plan (no collectives):
- Embedding gather, input-gate projection (xg) and the LSTM recurrence are
  replicated on every core (the recurrence is inherently sequential and
  batch=64 limits useful intra-step parallelism).
- The output projection (the dominant 537 GFLOP) is vocab-sharded
  column-parallel: core j computes logits[:, j*4000:(j+1)*4000].
- Matmuls run in float32r (TF32-class, 1 cycle/row for moving dim >= 256);
  elementwise stays fp32.

Layouts:
- Recurrence is h-stationary: gates[64, 4096] += hT_k[128,64].T @ W_hh_k[128,512]
  accumulated over 8 k-tiles in PSUM, one 512-column bank at a time.
- hT (transposed hidden) is produced each step via 8 PE transposes; it feeds
  the next step's stationary operand and is streamed to DRAM as the
  projection's stationary operand [128 h, 128 tok] tiles.
"""

import sys

sys.path.insert(0, "/opt/trn_rl_repo")

import numpy as np

P = 128
SEQ = 128
BATCH = 64
EMB = 512
HID = 1024
VOCAB = 32000
G4 = 4 * HID            # 4096
TOK = SEQ * BATCH       # 8192
NT = TOK // P           # 64 token tiles
N_CORES = 8
VSH = VOCAB // N_CORES  # 4000 vocab shard per core
VC = 500                # projection moving-dim chunk (<=512, >=256 for f32r)
NVC = VSH // VC         # 8
KE = EMB // P           # 4 contraction tiles for xg
KH = HID // P           # 8 contraction tiles for gates/projection
NG = G4 // 512          # 8 gate column chunks
XGC = 8                 # xg DRAM scratch time-chunks
HC = 4                  # hT DRAM scratch time-chunks


def _build_program(phases="ABC"):
    import os
    import concourse.bacc as bacc
    import concourse.bass as bass
    import concourse.mybir as mybir
    from concourse import tile
    from concourse.masks import make_identity

    f32 = mybir.dt.float32
    f32r = mybir.dt.float32r
    i32 = mybir.dt.int32
    AF = mybir.ActivationFunctionType
    ALU = mybir.AluOpType

    nc = bacc.Bacc("TRN2", target_bir_lowering=False, debug=False,
                   num_devices=N_CORES)

    idx_d = nc.dram_tensor("idx", [TOK, 1], i32, kind="ExternalInput")
    emb_d = nc.dram_tensor("emb", [VOCAB, EMB], f32, kind="ExternalInput")
    wih_d = nc.dram_tensor("wih", [EMB, G4], f32r, kind="ExternalInput")
    whh_d = nc.dram_tensor("whh", [HID, G4], f32r, kind="ExternalInput")
    brep_d = nc.dram_tensor("brep", [P, G4], f32, kind="ExternalInput")
    lwt_d = nc.dram_tensor("lwt", [HID, VSH], f32r, kind="ExternalInput")
    lbrep_d = nc.dram_tensor("lbrep", [P, VSH], f32, kind="ExternalInput")
    h0_d = nc.dram_tensor("h0", [BATCH, HID], f32, kind="ExternalInput")
    c0_d = nc.dram_tensor("c0", [BATCH, HID], f32, kind="ExternalInput")

    logits_d = nc.dram_tensor("logits", [TOK, VSH], f32, kind="ExternalOutput")
    ht_d = nc.dram_tensor("ht", [BATCH, HID], f32, kind="ExternalOutput")
    ct_d = nc.dram_tensor("ct", [BATCH, HID], f32, kind="ExternalOutput")

    with tile.TileContext(nc) as tc:
        with (
            tc.tile_pool(name="dram", bufs=1, space="DRAM") as dram,
            tc.tile_pool(name="const", bufs=1) as constp,
        ):
            # DRAM scratch, split into time-chunks so phase seams overlap
            # (whole-tensor deps would serialize A->B->C completely).
            xg_d = [dram.tile([TOK // XGC, G4], f32, tag=f"xg{g}", name=f"xg{g}")
                    for g in range(XGC)]
            # hT_seq: per k-tile, HC time-chunks (f32r)
            hts_d = [[dram.tile([P, TOK // HC], f32r, tag=f"hts{k}_{g}",
                                name=f"hts{k}_{g}") for g in range(HC)]
                     for k in range(KH)]

            ident = constp.tile([P, P], f32)
            make_identity(nc, ident[:])

            # ---------------- Phase A: gather + xg = x @ W_ih + b -------------
            if "A" in phases:
              with (
                tc.tile_pool(name="wih", bufs=1) as wihp,
                tc.tile_pool(name="bias", bufs=1) as biasp,
                tc.tile_pool(name="idx", bufs=3) as idxp,
                tc.tile_pool(name="xa", bufs=3) as xap,
                tc.tile_pool(name="xtp", bufs=3, space="PSUM") as xtps,
                tc.tile_pool(name="xts", bufs=3) as xtsb,
                tc.tile_pool(name="xgp", bufs=5, space="PSUM") as xgps,
                tc.tile_pool(name="xge", bufs=6) as xgev,
            ):
                wih = wihp.tile([P, KE * G4], f32r)
                wih_r = wih.rearrange("p (k g) -> k p g", k=KE)
                wihd_r = wih_d.ap().rearrange("(k p) g -> k p g", p=P)
                for k in range(KE):
                    nc.sync.dma_start(wih_r[k], wihd_r[k])
                brep = biasp.tile([P, G4], f32)
                nc.sync.dma_start(brep[:], brep_d.ap()[:])

                for t in range(NT):
                    idxt = idxp.tile([P, 1], i32)
                    nc.sync.dma_start(idxt[:], idx_d.ap()[t * P:(t + 1) * P, :])
                    xt = xap.tile([P, EMB], f32)
                    nc.gpsimd.indirect_dma_start(
                        out=xt[:],
                        out_offset=None,
                        in_=emb_d.ap()[:],
                        in_offset=bass.IndirectOffsetOnAxis(ap=idxt[:, :1], axis=0),
                    )
                    xTt = []
                    for e in range(KE):
                        pst = xtps.tile([P, P], f32)
                        nc.tensor.transpose(pst[:], xt[:, e * P:(e + 1) * P], ident[:])
                        xte = xtsb.tile([P, P], f32r, tag=f"xT{e}", name=f"xT{e}")
                        nc.vector.tensor_copy(xte[:], pst[:])
                        xTt.append(xte)
                    for n in range(NG):
                        ps = xgps.tile([P, 512], f32)
                        for k in range(KE):
                            nc.tensor.matmul(
                                ps[:], xTt[k][:], wih_r[k][:, n * 512:(n + 1) * 512],
                                start=(k == 0), stop=(k == KE - 1),
                            )
                        sb = xgev.tile([P, 512], f32)
                        nc.vector.tensor_tensor(
                            sb[:], ps[:], brep[:, n * 512:(n + 1) * 512], op=ALU.add)
                        nc.sync.dma_start(
                            xg_d[t // (NT // XGC)][
                                (t % (NT // XGC)) * P:(t % (NT // XGC)) * P + P,
                                n * 512:(n + 1) * 512], sb[:])

            # ---------------- Phase B: LSTM recurrence ------------------------
            if "B" in phases:
              with (
                tc.tile_pool(name="whh", bufs=1) as whhp,
                tc.tile_pool(name="xgin", bufs=3) as xginp,
                tc.tile_pool(name="gps", bufs=6, space="PSUM") as gpsp,
                tc.tile_pool(name="pre", bufs=3) as prep,
                tc.tile_pool(name="gc", bufs=2) as gcp,
                tc.tile_pool(name="state", bufs=1) as statep,
                tc.tile_pool(name="tmp", bufs=1) as tmpp,
                tc.tile_pool(name="htp", bufs=2, space="PSUM") as htps,
                tc.tile_pool(name="hts", bufs=2) as htsb,
            ):
                whh = whhp.tile([P, KH * G4], f32r)
                whh_r = whh.rearrange("p (k g) -> k p g", k=KH)
                whhd_r = whh_d.ap().rearrange("(k p) g -> k p g", p=P)
                for k in range(KH):
                    nc.sync.dma_start(whh_r[k], whhd_r[k])

                cbuf = statep.tile([BATCH, HID], f32, tag="c")
                hbuf = statep.tile([BATCH, HID], f32, tag="h")
                nc.sync.dma_start(cbuf[:], c0_d.ap()[:])
                nc.sync.dma_start(hbuf[:], h0_d.ap()[:])

                def transpose_h(src):
                    tiles = []
                    for e in range(KH):
                        pst = htps.tile([P, BATCH], f32)
                        nc.tensor.transpose(
                            pst[:], src[:, e * P:(e + 1) * P], ident[:BATCH, :BATCH])
                        hte = htsb.tile([P, BATCH], f32r, tag=f"hT{e}", name=f"hT{e}")
                        nc.vector.tensor_copy(hte[:], pst[:])
                        tiles.append(hte)
                    return tiles

                hT = transpose_h(hbuf)

                # Gate chunk n covers gates[:, n*512:(n+1)*512]; h-slice e
                # (128 cols of H) needs chunks {e//4, 2+e//4, 4+e//4, 6+e//4}.
                # Emit chunks 0,2,4,6 first so h-slices 0..3 (and their
                # transposes feeding the next step) complete while the PE is
                # still streaming chunks 1,3,5,7.
                n_order = [0, 2, 4, 6, 1, 3, 5, 7]
                for s in range(SEQ):
                    gc = [None] * NG
                    for n in n_order:
                        xgs = xginp.tile([BATCH, 512], f32, tag="xgs", name="xgs")
                        nc.sync.dma_start(
                            xgs[:],
                            xg_d[s // (SEQ // XGC)][
                                (s % (SEQ // XGC)) * BATCH:
                                (s % (SEQ // XGC)) * BATCH + BATCH,
                                n * 512:(n + 1) * 512])
                        ps = gpsp.tile([BATCH, 512], f32)
                        for k in range(KH):
                            nc.tensor.matmul(
                                ps[:], hT[k][:], whh_r[k][:, n * 512:(n + 1) * 512],
                                start=(k == 0), stop=(k == KH - 1),
                            )
                        pre = prep.tile([BATCH, 512], f32)
                        nc.vector.tensor_tensor(pre[:], ps[:], xgs[:], op=ALU.add)
                        func = AF.Tanh if n in (4, 5) else AF.Sigmoid
                        gcn = gcp.tile([BATCH, 512], f32, tag=f"gc{n}", name=f"gc{n}")
                        nc.scalar.activation(gcn[:], pre[:], func)
                        gc[n] = gcn
                    hT = []
                    for e in range(KH):
                        sl = slice((e % 4) * P, (e % 4) * P + P)
                        icn, fcn = gc[e // 4], gc[2 + e // 4]
                        gcn, ocn = gc[4 + e // 4], gc[6 + e // 4]
                        csl = cbuf[:, e * P:(e + 1) * P]
                        hsl = hbuf[:, e * P:(e + 1) * P]
                        t1 = tmpp.tile([BATCH, P], f32, tag=f"t1_{e}", name=f"t1_{e}")
                        nc.vector.tensor_tensor(t1[:], fcn[:, sl], csl, op=ALU.mult)
                        nc.vector.tensor_tensor(csl, icn[:, sl], gcn[:, sl], op=ALU.mult)
                        nc.vector.tensor_tensor(csl, t1[:], csl, op=ALU.add)
                        nc.scalar.activation(t1[:], csl, AF.Tanh)
                        nc.vector.tensor_tensor(hsl, ocn[:, sl], t1[:], op=ALU.mult)
                        pst = htps.tile([P, BATCH], f32)
                        nc.tensor.transpose(pst[:], hsl, ident[:BATCH, :BATCH])
                        hte = htsb.tile([P, BATCH], f32r, tag=f"hT{e}", name=f"hT{e}")
                        nc.vector.tensor_copy(hte[:], pst[:])
                        nc.sync.dma_start(
                            hts_d[e][s // (SEQ // HC)][
                                :, (s % (SEQ // HC)) * BATCH:
                                (s % (SEQ // HC)) * BATCH + BATCH], hte[:])
                        hT.append(hte)

                nc.sync.dma_start(ht_d.ap()[:], hbuf[:])
                nc.sync.dma_start(ct_d.ap()[:], cbuf[:])

            # ---------------- Phase C: projection -----------------------------
            if "C" in phases:
              with (
                tc.tile_pool(name="lw", bufs=1) as lwp,
                tc.tile_pool(name="lb", bufs=1) as lbp,
                tc.tile_pool(name="hst", bufs=4) as hstp,
                tc.tile_pool(name="pps", bufs=6, space="PSUM") as ppsp,
                tc.tile_pool(name="pev", bufs=6) as pevp,
            ):
                lw = lwp.tile([P, KH * VSH], f32r)
                lw_r = lw.rearrange("p (k v) -> k p v", k=KH)
                lwd_r = lwt_d.ap().rearrange("(k p) v -> k p v", p=P)
                for k in range(KH):
                    nc.sync.dma_start(lw_r[k], lwd_r[k])
                lbrep = lbp.tile([P, VSH], f32)
                nc.sync.dma_start(lbrep[:], lbrep_d.ap()[:])

                for t in range(NT):
                    hTt = []
                    for k in range(KH):
                        htk = hstp.tile([P, P], f32r, tag=f"pht{k}", name=f"pht{k}")
                        nc.sync.dma_start(
                            htk[:],
                            hts_d[k][t // (NT // HC)][
                                :, (t % (NT // HC)) * P:(t % (NT // HC)) * P + P])
                        hTt.append(htk)
                    for v in range(NVC):
                        ps = ppsp.tile([P, VC], f32)
                        for k in range(KH):
                            nc.tensor.matmul(
                                ps[:], hTt[k][:], lw_r[k][:, v * VC:(v + 1) * VC],
                                start=(k == 0), stop=(k == KH - 1),
                            )
                        sb = pevp.tile([P, VC], f32)
                        nc.vector.tensor_tensor(
                            sb[:], ps[:], lbrep[:, v * VC:(v + 1) * VC], op=ALU.add)
                        nc.sync.dma_start(
                            logits_d.ap()[t * P:(t + 1) * P, v * VC:(v + 1) * VC],
                            sb[:])

    nc.compile()
    return nc


_NC = None


def _get_program():
    global _NC
    if _NC is None:
        _NC = _build_program()
    return _NC


def kernel(**inputs):
    from concourse.bass_utils import run_bass_kernel_spmd

    inp = np.asarray(inputs["input"])
    emb = np.ascontiguousarray(np.asarray(inputs["emb"], dtype=np.float32))
    W_ih = np.ascontiguousarray(np.asarray(inputs["W_ih"], dtype=np.float32))
    W_hh = np.ascontiguousarray(np.asarray(inputs["W_hh"], dtype=np.float32))
    b_lstm = np.asarray(inputs["b_lstm"], dtype=np.float32)
    lin_W = np.asarray(inputs["lin_W"], dtype=np.float32)
    lin_b = np.asarray(inputs["lin_b"], dtype=np.float32)
    h0 = np.ascontiguousarray(np.asarray(inputs["h0"], dtype=np.float32))
    c0 = np.ascontiguousarray(np.asarray(inputs["c0"], dtype=np.float32))

    idx = np.ascontiguousarray(inp.reshape(TOK, 1).astype(np.int32))
    brep = np.ascontiguousarray(np.broadcast_to(b_lstm[None, :], (P, G4)).astype(np.float32))
    linWT = np.ascontiguousarray(lin_W.T)  # [HID, VOCAB]

    in_maps = []
    for j in range(N_CORES):
        vs = slice(j * VSH, (j + 1) * VSH)
        in_maps.append({
            "idx": idx,
            "emb": emb,
            "wih": W_ih,
            "whh": W_hh,
            "brep": brep,
            "lwt": np.ascontiguousarray(linWT[:, vs]),
            "lbrep": np.ascontiguousarray(
                np.broadcast_to(lin_b[None, vs], (P, VSH)).astype(np.float32)),
            "h0": h0,
            "c0": c0,
        })

    nc = _get_program()
    res = run_bass_kernel_spmd(nc, in_maps, core_ids=list(range(N_CORES)))

    logits = np.concatenate(
        [res.results[j]["logits"] for j in range(N_CORES)], axis=1)
    ht = res.results[0]["ht"]
    ct = res.results[0]["ct"]
    return logits, (ht, ct)


# revision 13
# speedup vs baseline: 1.0554x; 1.0554x over previous
"""Trainium2 Bass kernel for an LSTM poetry model.

Model: x = emb[input]; xg = x @ W_ih + b; LSTM scan over 128 steps;
logits = hidden @ lin_W.T + lin_b.

8-core SPMD plan (no collectives):
- Embedding gather, input-gate projection (xg) and the LSTM recurrence are
  replicated on every core (the recurrence is inherently sequential and
  batch=64 limits useful intra-step parallelism).
- The output projection (the dominant 537 GFLOP) is vocab-sharded
  column-parallel: core j computes logits[:, j*4000:(j+1)*4000].
- Matmuls run in float32r (TF32-class, 1 cycle/row for moving dim >= 256);
  elementwise stays fp32.

Layouts:
- Recurrence is h-stationary: gates[64, 4096] += hT_k[128,64].T @ W_hh_k[128,512]
  accumulated over 8 k-tiles in PSUM, one 512-column bank at a time.
- hT (transposed hidden) is produced each step via 8 PE transposes; it feeds
  the next step's stationary operand and is streamed to DRAM as the
  projection's stationary operand [128 h, 128 tok] tiles.
"""

import sys

sys.path.insert(0, "/opt/trn_rl_repo")

import numpy as np

P = 128
SEQ = 128
BATCH = 64
EMB = 512
HID = 1024
VOCAB = 32000
G4 = 4 * HID            # 4096
TOK = SEQ * BATCH       # 8192
NT = TOK // P           # 64 token tiles
N_CORES = 8
VSH = VOCAB // N_CORES  # 4000 vocab shard per core
VC = 500                # projection moving-dim chunk (<=512, >=256 for f32r)
NVC = VSH // VC         # 8
KE = EMB // P           # 4 contraction tiles for xg
KH = HID // P           # 8 contraction tiles for gates/projection
NG = G4 // 512          # 8 gate column chunks
XGC = 8                 # xg DRAM scratch time-chunks
HC = 4                  # hT DRAM scratch time-chunks


def _build_program(phases="ABC"):
    import os
    import concourse.bacc as bacc
    import concourse.bass as bass
    import concourse.mybir as mybir
    from concourse import tile
    from concourse.masks import make_identity

    f32 = mybir.dt.float32
    f32r = mybir.dt.float32r
    i32 = mybir.dt.int32
    AF = mybir.ActivationFunctionType
    ALU = mybir.AluOpType

    nc = bacc.Bacc("TRN2", target_bir_lowering=False, debug=False,
                   num_devices=N_CORES)

    idx_d = nc.dram_tensor("idx", [TOK, 1], i32, kind="ExternalInput")
    emb_d = nc.dram_tensor("emb", [VOCAB, EMB], f32, kind="ExternalInput")
    wih_d = nc.dram_tensor("wih", [EMB, G4], f32r, kind="ExternalInput")
    whh_d = nc.dram_tensor("whh", [HID, G4], f32r, kind="ExternalInput")
    brep_d = nc.dram_tensor("brep", [P, G4], f32, kind="ExternalInput")
    lwt_d = nc.dram_tensor("lwt", [HID, VSH], f32r, kind="ExternalInput")
    lbrep_d = nc.dram_tensor("lbrep", [P, VSH], f32, kind="ExternalInput")
    h0_d = nc.dram_tensor("h0", [BATCH, HID], f32, kind="ExternalInput")
    c0_d = nc.dram_tensor("c0", [BATCH, HID], f32, kind="ExternalInput")

    logits_d = nc.dram_tensor("logits", [TOK, VSH], f32, kind="ExternalOutput")
    ht_d = nc.dram_tensor("ht", [BATCH, HID], f32, kind="ExternalOutput")
    ct_d = nc.dram_tensor("ct", [BATCH, HID], f32, kind="ExternalOutput")

    with tile.TileContext(nc) as tc:
        with (
            tc.tile_pool(name="dram", bufs=1, space="DRAM") as dram,
            tc.tile_pool(name="const", bufs=1) as constp,
        ):
            # DRAM scratch, split into time-chunks so phase seams overlap
            # (whole-tensor deps would serialize A->B->C completely).
            xg_d = [dram.tile([TOK // XGC, G4], f32, tag=f"xg{g}", name=f"xg{g}")
                    for g in range(XGC)]
            # hT_seq: per k-tile, HC time-chunks (f32r)
            hts_d = [[dram.tile([P, TOK // HC], f32r, tag=f"hts{k}_{g}",
                                name=f"hts{k}_{g}") for g in range(HC)]
                     for k in range(KH)]

            ident = constp.tile([P, P], f32)
            make_identity(nc, ident[:])

            # ---------------- Phase A: gather + xg = x @ W_ih + b -------------
            if "A" in phases:
              with (
                tc.tile_pool(name="wih", bufs=1) as wihp,
                tc.tile_pool(name="bias", bufs=1) as biasp,
                tc.tile_pool(name="idx", bufs=3) as idxp,
                tc.tile_pool(name="xa", bufs=3) as xap,
                tc.tile_pool(name="xtp", bufs=3, space="PSUM") as xtps,
                tc.tile_pool(name="xts", bufs=3) as xtsb,
                tc.tile_pool(name="xgp", bufs=5, space="PSUM") as xgps,
                tc.tile_pool(name="xge", bufs=6) as xgev,
            ):
                wih = wihp.tile([P, KE * G4], f32r)
                wih_r = wih.rearrange("p (k g) -> k p g", k=KE)
                wihd_r = wih_d.ap().rearrange("(k p) g -> k p g", p=P)
                for k in range(KE):
                    nc.sync.dma_start(wih_r[k], wihd_r[k])
                brep = biasp.tile([P, G4], f32)
                nc.sync.dma_start(brep[:], brep_d.ap()[:])

                for t in range(NT):
                    idxt = idxp.tile([P, 1], i32)
                    nc.sync.dma_start(idxt[:], idx_d.ap()[t * P:(t + 1) * P, :])
                    xt = xap.tile([P, EMB], f32)
                    nc.gpsimd.indirect_dma_start(
                        out=xt[:],
                        out_offset=None,
                        in_=emb_d.ap()[:],
                        in_offset=bass.IndirectOffsetOnAxis(ap=idxt[:, :1], axis=0),
                    )
                    xTt = []
                    for e in range(KE):
                        pst = xtps.tile([P, P], f32)
                        nc.tensor.transpose(pst[:], xt[:, e * P:(e + 1) * P], ident[:])
                        xte = xtsb.tile([P, P], f32r, tag=f"xT{e}", name=f"xT{e}")
                        nc.vector.tensor_copy(xte[:], pst[:])
                        xTt.append(xte)
                    for n in range(NG):
                        ps = xgps.tile([P, 512], f32)
                        for k in range(KE):
                            nc.tensor.matmul(
                                ps[:], xTt[k][:], wih_r[k][:, n * 512:(n + 1) * 512],
                                start=(k == 0), stop=(k == KE - 1),
                            )
                        sb = xgev.tile([P, 512], f32)
                        nc.vector.tensor_tensor(
                            sb[:], ps[:], brep[:, n * 512:(n + 1) * 512], op=ALU.add)
                        nc.sync.dma_start(
                            xg_d[t // (NT // XGC)][
                                (t % (NT // XGC)) * P:(t % (NT // XGC)) * P + P,
                                n * 512:(n + 1) * 512], sb[:])

            # ---------------- Phase B: LSTM recurrence ------------------------
            if "B" in phases:
              with (
                tc.tile_pool(name="whh", bufs=1) as whhp,
                tc.tile_pool(name="xgin", bufs=3) as xginp,
                tc.tile_pool(name="gps", bufs=6, space="PSUM") as gpsp,
                tc.tile_pool(name="pre", bufs=3) as prep,
                tc.tile_pool(name="gc", bufs=2) as gcp,
                tc.tile_pool(name="state", bufs=1) as statep,
                tc.tile_pool(name="tmp", bufs=1) as tmpp,
                tc.tile_pool(name="htp", bufs=2, space="PSUM") as htps,
                tc.tile_pool(name="hts", bufs=2) as htsb,
            ):
                whh = whhp.tile([P, KH * G4], f32r)
                whh_r = whh.rearrange("p (k g) -> k p g", k=KH)
                whhd_r = whh_d.ap().rearrange("(k p) g -> k p g", p=P)
                for k in range(KH):
                    nc.sync.dma_start(whh_r[k], whhd_r[k])

                cbuf = statep.tile([BATCH, HID], f32, tag="c")
                hbuf = statep.tile([BATCH, HID], f32, tag="h")
                nc.sync.dma_start(cbuf[:], c0_d.ap()[:])
                nc.sync.dma_start(hbuf[:], h0_d.ap()[:])

                def transpose_h(src):
                    tiles = []
                    for e in range(KH):
                        pst = htps.tile([P, BATCH], f32)
                        nc.tensor.transpose(
                            pst[:], src[:, e * P:(e + 1) * P], ident[:BATCH, :BATCH])
                        hte = htsb.tile([P, BATCH], f32r, tag=f"hT{e}", name=f"hT{e}")
                        nc.vector.tensor_copy(hte[:], pst[:])
                        tiles.append(hte)
                    return tiles

                hT = transpose_h(hbuf)

                # Gate chunk n covers gates[:, n*512:(n+1)*512]; h-slice e
                # (128 cols of H) needs chunks {e//4, 2+e//4, 4+e//4, 6+e//4}.
                # Emit chunks 0,2,4,6 first so h-slices 0..3 (and their
                # transposes feeding the next step) complete while the PE is
                # still streaming chunks 1,3,5,7.
                for s in range(SEQ):
                    gc = [None] * NG
                    hT_new = []

                    def emit_chunk(n):
                        xgs = xginp.tile([BATCH, 512], f32, tag="xgs", name="xgs")
                        nc.sync.dma_start(
                            xgs[:],
                            xg_d[s // (SEQ // XGC)][
                                (s % (SEQ // XGC)) * BATCH:
                                (s % (SEQ // XGC)) * BATCH + BATCH,
                                n * 512:(n + 1) * 512])
                        ps = gpsp.tile([BATCH, 512], f32)
                        for k in range(KH):
                            nc.tensor.matmul(
                                ps[:], hT[k][:], whh_r[k][:, n * 512:(n + 1) * 512],
                                start=(k == 0), stop=(k == KH - 1),
                            )
                        pre = prep.tile([BATCH, 512], f32)
                        nc.vector.tensor_tensor(pre[:], ps[:], xgs[:], op=ALU.add)
                        func = AF.Tanh if n in (4, 5) else AF.Sigmoid
                        gcn = gcp.tile([BATCH, 512], f32, tag=f"gc{n}", name=f"gc{n}")
                        nc.scalar.activation(gcn[:], pre[:], func)
                        gc[n] = gcn

                    def emit_slice(e):
                        sl = slice((e % 4) * P, (e % 4) * P + P)
                        icn, fcn = gc[e // 4], gc[2 + e // 4]
                        gcn, ocn = gc[4 + e // 4], gc[6 + e // 4]
                        csl = cbuf[:, e * P:(e + 1) * P]
                        hsl = hbuf[:, e * P:(e + 1) * P]
                        t1 = tmpp.tile([BATCH, P], f32, tag=f"t1_{e}", name=f"t1_{e}")
                        nc.vector.tensor_tensor(t1[:], fcn[:, sl], csl, op=ALU.mult)
                        nc.vector.tensor_tensor(csl, icn[:, sl], gcn[:, sl], op=ALU.mult)
                        nc.vector.tensor_tensor(csl, t1[:], csl, op=ALU.add)
                        nc.scalar.activation(t1[:], csl, AF.Tanh)
                        nc.vector.tensor_tensor(hsl, ocn[:, sl], t1[:], op=ALU.mult)
                        pst = htps.tile([P, BATCH], f32)
                        nc.tensor.transpose(pst[:], hsl, ident[:BATCH, :BATCH])
                        hte = htsb.tile([P, BATCH], f32r, tag=f"hT{e}", name=f"hT{e}")
                        nc.vector.tensor_copy(hte[:], pst[:])
                        nc.sync.dma_start(
                            hts_d[e][s // (SEQ // HC)][
                                :, (s % (SEQ // HC)) * BATCH:
                                (s % (SEQ // HC)) * BATCH + BATCH], hte[:])
                        hT_new.append(hte)

                    # Emit gate chunks 0,2,4,6 then h-slices 0-3, then chunks
                    # 1,3,5,7 and h-slices 4-7, so each half-step's elementwise
                    # and transposes interleave with the other half's matmuls.
                    for n in (0, 2, 4, 6):
                        emit_chunk(n)
                    for e in range(KH // 2):
                        emit_slice(e)
                    for n in (1, 3, 5, 7):
                        emit_chunk(n)
                    for e in range(KH // 2, KH):
                        emit_slice(e)
                    hT = hT_new

                nc.sync.dma_start(ht_d.ap()[:], hbuf[:])
                nc.sync.dma_start(ct_d.ap()[:], cbuf[:])

            # ---------------- Phase C: projection -----------------------------
            if "C" in phases:
              with (
                tc.tile_pool(name="lw", bufs=1) as lwp,
                tc.tile_pool(name="lb", bufs=1) as lbp,
                tc.tile_pool(name="hst", bufs=4) as hstp,
                tc.tile_pool(name="pps", bufs=6, space="PSUM") as ppsp,
                tc.tile_pool(name="pev", bufs=6) as pevp,
            ):
                lw = lwp.tile([P, KH * VSH], f32r)
                lw_r = lw.rearrange("p (k v) -> k p v", k=KH)
                lwd_r = lwt_d.ap().rearrange("(k p) v -> k p v", p=P)
                for k in range(KH):
                    nc.sync.dma_start(lw_r[k], lwd_r[k])
                lbrep = lbp.tile([P, VSH], f32)
                nc.sync.dma_start(lbrep[:], lbrep_d.ap()[:])

                for t in range(NT):
                    hTt = []
                    for k in range(KH):
                        htk = hstp.tile([P, P], f32r, tag=f"pht{k}", name=f"pht{k}")
                        nc.sync.dma_start(
                            htk[:],
                            hts_d[k][t // (NT // HC)][
                                :, (t % (NT // HC)) * P:(t % (NT // HC)) * P + P])
                        hTt.append(htk)
                    for v in range(NVC):
                        ps = ppsp.tile([P, VC], f32)
                        for k in range(KH):
                            nc.tensor.matmul(
                                ps[:], hTt[k][:], lw_r[k][:, v * VC:(v + 1) * VC],
                                start=(k == 0), stop=(k == KH - 1),
                            )
                        sb = pevp.tile([P, VC], f32)
                        nc.vector.tensor_tensor(
                            sb[:], ps[:], lbrep[:, v * VC:(v + 1) * VC], op=ALU.add)
                        nc.sync.dma_start(
                            logits_d.ap()[t * P:(t + 1) * P, v * VC:(v + 1) * VC],
                            sb[:])

    nc.compile()
    return nc


_NC = None


def _get_program():
    global _NC
    if _NC is None:
        _NC = _build_program()
    return _NC


def kernel(**inputs):
    from concourse.bass_utils import run_bass_kernel_spmd

    inp = np.asarray(inputs["input"])
    emb = np.ascontiguousarray(np.asarray(inputs["emb"], dtype=np.float32))
    W_ih = np.ascontiguousarray(np.asarray(inputs["W_ih"], dtype=np.float32))
    W_hh = np.ascontiguousarray(np.asarray(inputs["W_hh"], dtype=np.float32))
    b_lstm = np.asarray(inputs["b_lstm"], dtype=np.float32)
    lin_W = np.asarray(inputs["lin_W"], dtype=np.float32)
    lin_b = np.asarray(inputs["lin_b"], dtype=np.float32)
    h0 = np.ascontiguousarray(np.asarray(inputs["h0"], dtype=np.float32))
    c0 = np.ascontiguousarray(np.asarray(inputs["c0"], dtype=np.float32))

    idx = np.ascontiguousarray(inp.reshape(TOK, 1).astype(np.int32))
    brep = np.ascontiguousarray(np.broadcast_to(b_lstm[None, :], (P, G4)).astype(np.float32))
    linWT = np.ascontiguousarray(lin_W.T)  # [HID, VOCAB]

    in_maps = []
    for j in range(N_CORES):
        vs = slice(j * VSH, (j + 1) * VSH)
        in_maps.append({
            "idx": idx,
            "emb": emb,
            "wih": W_ih,
            "whh": W_hh,
            "brep": brep,
            "lwt": np.ascontiguousarray(linWT[:, vs]),
            "lbrep": np.ascontiguousarray(
                np.broadcast_to(lin_b[None, vs], (P, VSH)).astype(np.float32)),
            "h0": h0,
            "c0": c0,
        })

    nc = _get_program()
    res = run_bass_kernel_spmd(nc, in_maps, core_ids=list(range(N_CORES)))

    logits = np.concatenate(
        [res.results[j]["logits"] for j in range(N_CORES)], axis=1)
    ht = res.results[0]["ht"]
    ct = res.results[0]["ct"]
    return logits, (ht, ct)


# revision 14
# speedup vs baseline: 1.0574x; 1.0019x over previous
"""Trainium2 Bass kernel for an LSTM poetry model.

Model: x = emb[input]; xg = x @ W_ih + b; LSTM scan over 128 steps;
logits = hidden @ lin_W.T + lin_b.

8-core SPMD plan (no collectives):
- Embedding gather, input-gate projection (xg) and the LSTM recurrence are
  replicated on every core (the recurrence is inherently sequential and
  batch=64 limits useful intra-step parallelism).
- The output projection (the dominant 537 GFLOP) is vocab-sharded
  column-parallel: core j computes logits[:, j*4000:(j+1)*4000].
- Matmuls run in float32r (TF32-class, 1 cycle/row for moving dim >= 256);
  elementwise stays fp32.

Layouts:
- Recurrence is h-stationary: gates[64, 4096] += hT_k[128,64].T @ W_hh_k[128,512]
  accumulated over 8 k-tiles in PSUM, one 512-column bank at a time.
- hT (transposed hidden) is produced each step via 8 PE transposes; it feeds
  the next step's stationary operand and is streamed to DRAM as the
  projection's stationary operand [128 h, 128 tok] tiles.
"""

import sys

sys.path.insert(0, "/opt/trn_rl_repo")

import numpy as np

P = 128
SEQ = 128
BATCH = 64
EMB = 512
HID = 1024
VOCAB = 32000
G4 = 4 * HID            # 4096
TOK = SEQ * BATCH       # 8192
NT = TOK // P           # 64 token tiles
N_CORES = 8
VSH = VOCAB // N_CORES  # 4000 vocab shard per core
VC = 500                # projection moving-dim chunk (<=512, >=256 for f32r)
NVC = VSH // VC         # 8
KE = EMB // P           # 4 contraction tiles for xg
KH = HID // P           # 8 contraction tiles for gates/projection
NG = G4 // 512          # 8 gate column chunks
XGC = 8                 # xg DRAM scratch time-chunks
HC = 4                  # hT DRAM scratch time-chunks


def _build_program(phases="ABC"):
    import os
    import concourse.bacc as bacc
    import concourse.bass as bass
    import concourse.mybir as mybir
    from concourse import tile
    from concourse.masks import make_identity

    f32 = mybir.dt.float32
    f32r = mybir.dt.float32r
    i32 = mybir.dt.int32
    AF = mybir.ActivationFunctionType
    ALU = mybir.AluOpType

    nc = bacc.Bacc("TRN2", target_bir_lowering=False, debug=False,
                   num_devices=N_CORES)

    idx_d = nc.dram_tensor("idx", [TOK, 1], i32, kind="ExternalInput")
    emb_d = nc.dram_tensor("emb", [VOCAB, EMB], f32, kind="ExternalInput")
    wih_d = nc.dram_tensor("wih", [EMB, G4], f32r, kind="ExternalInput")
    whh_d = nc.dram_tensor("whh", [HID, G4], f32r, kind="ExternalInput")
    brep_d = nc.dram_tensor("brep", [P, G4], f32, kind="ExternalInput")
    lwt_d = nc.dram_tensor("lwt", [HID, VSH], f32r, kind="ExternalInput")
    lbrep_d = nc.dram_tensor("lbrep", [P, VSH], f32, kind="ExternalInput")
    h0_d = nc.dram_tensor("h0", [BATCH, HID], f32, kind="ExternalInput")
    c0_d = nc.dram_tensor("c0", [BATCH, HID], f32, kind="ExternalInput")

    logits_d = nc.dram_tensor("logits", [TOK, VSH], f32, kind="ExternalOutput")
    ht_d = nc.dram_tensor("ht", [BATCH, HID], f32, kind="ExternalOutput")
    ct_d = nc.dram_tensor("ct", [BATCH, HID], f32, kind="ExternalOutput")

    with tile.TileContext(nc) as tc:
        with (
            tc.tile_pool(name="dram", bufs=1, space="DRAM") as dram,
            tc.tile_pool(name="const", bufs=1) as constp,
        ):
            # DRAM scratch, split into time-chunks so phase seams overlap
            # (whole-tensor deps would serialize A->B->C completely).
            xg_d = [dram.tile([TOK // XGC, G4], f32, tag=f"xg{g}", name=f"xg{g}")
                    for g in range(XGC)]
            # hT_seq: per k-tile, HC time-chunks (f32r)
            hts_d = [[dram.tile([P, TOK // HC], f32r, tag=f"hts{k}_{g}",
                                name=f"hts{k}_{g}") for g in range(HC)]
                     for k in range(KH)]

            ident = constp.tile([P, P], f32)
            make_identity(nc, ident[:])

            # ---------------- Phase A: gather + xg = x @ W_ih + b -------------
            if "A" in phases:
              with (
                tc.tile_pool(name="wih", bufs=1) as wihp,
                tc.tile_pool(name="bias", bufs=1) as biasp,
                tc.tile_pool(name="idx", bufs=3) as idxp,
                tc.tile_pool(name="xa", bufs=3) as xap,
                tc.tile_pool(name="xtp", bufs=3, space="PSUM") as xtps,
                tc.tile_pool(name="xts", bufs=3) as xtsb,
                tc.tile_pool(name="xgp", bufs=5, space="PSUM") as xgps,
                tc.tile_pool(name="xge", bufs=6) as xgev,
            ):
                wih = wihp.tile([P, KE * G4], f32r)
                wih_r = wih.rearrange("p (k g) -> k p g", k=KE)
                wihd_r = wih_d.ap().rearrange("(k p) g -> k p g", p=P)
                for k in range(KE):
                    nc.sync.dma_start(wih_r[k], wihd_r[k])
                brep = biasp.tile([P, G4], f32)
                nc.sync.dma_start(brep[:], brep_d.ap()[:])

                for t in range(NT):
                    idxt = idxp.tile([P, 1], i32)
                    nc.sync.dma_start(idxt[:], idx_d.ap()[t * P:(t + 1) * P, :])
                    xt = xap.tile([P, EMB], f32)
                    nc.gpsimd.indirect_dma_start(
                        out=xt[:],
                        out_offset=None,
                        in_=emb_d.ap()[:],
                        in_offset=bass.IndirectOffsetOnAxis(ap=idxt[:, :1], axis=0),
                    )
                    xTt = []
                    for e in range(KE):
                        pst = xtps.tile([P, P], f32)
                        nc.tensor.transpose(pst[:], xt[:, e * P:(e + 1) * P], ident[:])
                        xte = xtsb.tile([P, P], f32r, tag=f"xT{e}", name=f"xT{e}")
                        nc.vector.tensor_copy(xte[:], pst[:])
                        xTt.append(xte)
                    for n in range(NG):
                        ps = xgps.tile([P, 512], f32)
                        for k in range(KE):
                            nc.tensor.matmul(
                                ps[:], xTt[k][:], wih_r[k][:, n * 512:(n + 1) * 512],
                                start=(k == 0), stop=(k == KE - 1),
                            )
                        sb = xgev.tile([P, 512], f32)
                        nc.vector.tensor_tensor(
                            sb[:], ps[:], brep[:, n * 512:(n + 1) * 512], op=ALU.add)
                        nc.sync.dma_start(
                            xg_d[t // (NT // XGC)][
                                (t % (NT // XGC)) * P:(t % (NT // XGC)) * P + P,
                                n * 512:(n + 1) * 512], sb[:])

            # ---------------- Phase B: LSTM recurrence ------------------------
            if "B" in phases:
              with (
                tc.tile_pool(name="whh", bufs=1) as whhp,
                tc.tile_pool(name="xgin", bufs=3) as xginp,
                tc.tile_pool(name="gps", bufs=4, space="PSUM") as gpsp,
                tc.tile_pool(name="pre", bufs=3) as prep,
                tc.tile_pool(name="gc", bufs=2) as gcp,
                tc.tile_pool(name="state", bufs=1) as statep,
                tc.tile_pool(name="tmp", bufs=1) as tmpp,
                tc.tile_pool(name="htp", bufs=4, space="PSUM") as htps,
                tc.tile_pool(name="hts", bufs=2) as htsb,
            ):
                whh = whhp.tile([P, KH * G4], f32r)
                whh_r = whh.rearrange("p (k g) -> k p g", k=KH)
                whhd_r = whh_d.ap().rearrange("(k p) g -> k p g", p=P)
                for k in range(KH):
                    nc.sync.dma_start(whh_r[k], whhd_r[k])

                cbuf = statep.tile([BATCH, HID], f32, tag="c")
                hbuf = statep.tile([BATCH, HID], f32, tag="h")
                nc.sync.dma_start(cbuf[:], c0_d.ap()[:])
                nc.sync.dma_start(hbuf[:], h0_d.ap()[:])

                def transpose_h(src):
                    tiles = []
                    for e in range(KH):
                        pst = htps.tile([P, BATCH], f32)
                        nc.tensor.transpose(
                            pst[:], src[:, e * P:(e + 1) * P], ident[:BATCH, :BATCH])
                        hte = htsb.tile([P, BATCH], f32r, tag=f"hT{e}", name=f"hT{e}")
                        nc.vector.tensor_copy(hte[:], pst[:])
                        tiles.append(hte)
                    return tiles

                hT = transpose_h(hbuf)

                # Gate chunk n covers gates[:, n*512:(n+1)*512]; h-slice e
                # (128 cols of H) needs chunks {e//4, 2+e//4, 4+e//4, 6+e//4}.
                # Emit chunks 0,2,4,6 first so h-slices 0..3 (and their
                # transposes feeding the next step) complete while the PE is
                # still streaming chunks 1,3,5,7.
                for s in range(SEQ):
                    gc = [None] * NG
                    hT_new = []

                    def emit_chunk(n):
                        xgs = xginp.tile([BATCH, 512], f32, tag="xgs", name="xgs")
                        nc.sync.dma_start(
                            xgs[:],
                            xg_d[s // (SEQ // XGC)][
                                (s % (SEQ // XGC)) * BATCH:
                                (s % (SEQ // XGC)) * BATCH + BATCH,
                                n * 512:(n + 1) * 512])
                        ps = gpsp.tile([BATCH, 512], f32)
                        for k in range(KH):
                            nc.tensor.matmul(
                                ps[:], hT[k][:], whh_r[k][:, n * 512:(n + 1) * 512],
                                start=(k == 0), stop=(k == KH - 1),
                            )
                        pre = prep.tile([BATCH, 512], f32)
                        nc.vector.tensor_tensor(pre[:], ps[:], xgs[:], op=ALU.add)
                        func = AF.Tanh if n in (4, 5) else AF.Sigmoid
                        gcn = gcp.tile([BATCH, 512], f32, tag=f"gc{n}", name=f"gc{n}")
                        nc.scalar.activation(gcn[:], pre[:], func)
                        gc[n] = gcn

                    def emit_slice(e):
                        sl = slice((e % 4) * P, (e % 4) * P + P)
                        icn, fcn = gc[e // 4], gc[2 + e // 4]
                        gcn, ocn = gc[4 + e // 4], gc[6 + e // 4]
                        csl = cbuf[:, e * P:(e + 1) * P]
                        hsl = hbuf[:, e * P:(e + 1) * P]
                        t1 = tmpp.tile([BATCH, P], f32, tag=f"t1_{e}", name=f"t1_{e}")
                        nc.vector.tensor_tensor(t1[:], fcn[:, sl], csl, op=ALU.mult)
                        nc.vector.tensor_tensor(csl, icn[:, sl], gcn[:, sl], op=ALU.mult)
                        nc.vector.tensor_tensor(csl, t1[:], csl, op=ALU.add)
                        nc.scalar.activation(t1[:], csl, AF.Tanh)
                        nc.vector.tensor_tensor(hsl, ocn[:, sl], t1[:], op=ALU.mult)
                        pst = htps.tile([P, BATCH], f32)
                        nc.tensor.transpose(pst[:], hsl, ident[:BATCH, :BATCH])
                        hte = htsb.tile([P, BATCH], f32r, tag=f"hT{e}", name=f"hT{e}")
                        nc.vector.tensor_copy(hte[:], pst[:])
                        nc.sync.dma_start(
                            hts_d[e][s // (SEQ // HC)][
                                :, (s % (SEQ // HC)) * BATCH:
                                (s % (SEQ // HC)) * BATCH + BATCH], hte[:])
                        hT_new.append(hte)

                    # Emit gate chunks 0,2,4,6 then h-slices 0-3, then chunks
                    # 1,3,5,7 and h-slices 4-7, so each half-step's elementwise
                    # and transposes interleave with the other half's matmuls.
                    for n in (0, 2, 4, 6):
                        emit_chunk(n)
                    for e in range(KH // 2):
                        emit_slice(e)
                    for n in (1, 3, 5, 7):
                        emit_chunk(n)
                    for e in range(KH // 2, KH):
                        emit_slice(e)
                    hT = hT_new

                nc.sync.dma_start(ht_d.ap()[:], hbuf[:])
                nc.sync.dma_start(ct_d.ap()[:], cbuf[:])

            # ---------------- Phase C: projection -----------------------------
            if "C" in phases:
              with (
                tc.tile_pool(name="lw", bufs=1) as lwp,
                tc.tile_pool(name="lb", bufs=1) as lbp,
                tc.tile_pool(name="hst", bufs=4) as hstp,
                tc.tile_pool(name="pps", bufs=6, space="PSUM") as ppsp,
                tc.tile_pool(name="pev", bufs=6) as pevp,
            ):
                lw = lwp.tile([P, KH * VSH], f32r)
                lw_r = lw.rearrange("p (k v) -> k p v", k=KH)
                lwd_r = lwt_d.ap().rearrange("(k p) v -> k p v", p=P)
                for k in range(KH):
                    nc.sync.dma_start(lw_r[k], lwd_r[k])
                lbrep = lbp.tile([P, VSH], f32)
                nc.sync.dma_start(lbrep[:], lbrep_d.ap()[:])

                for t in range(NT):
                    hTt = []
                    for k in range(KH):
                        htk = hstp.tile([P, P], f32r, tag=f"pht{k}", name=f"pht{k}")
                        nc.sync.dma_start(
                            htk[:],
                            hts_d[k][t // (NT // HC)][
                                :, (t % (NT // HC)) * P:(t % (NT // HC)) * P + P])
                        hTt.append(htk)
                    for v in range(NVC):
                        ps = ppsp.tile([P, VC], f32)
                        for k in range(KH):
                            nc.tensor.matmul(
                                ps[:], hTt[k][:], lw_r[k][:, v * VC:(v + 1) * VC],
                                start=(k == 0), stop=(k == KH - 1),
                            )
                        sb = pevp.tile([P, VC], f32)
                        nc.vector.tensor_tensor(
                            sb[:], ps[:], lbrep[:, v * VC:(v + 1) * VC], op=ALU.add)
                        nc.sync.dma_start(
                            logits_d.ap()[t * P:(t + 1) * P, v * VC:(v + 1) * VC],
                            sb[:])

    nc.compile()
    return nc


_NC = None


def _get_program():
    global _NC
    if _NC is None:
        _NC = _build_program()
    return _NC


def kernel(**inputs):
    from concourse.bass_utils import run_bass_kernel_spmd

    inp = np.asarray(inputs["input"])
    emb = np.ascontiguousarray(np.asarray(inputs["emb"], dtype=np.float32))
    W_ih = np.ascontiguousarray(np.asarray(inputs["W_ih"], dtype=np.float32))
    W_hh = np.ascontiguousarray(np.asarray(inputs["W_hh"], dtype=np.float32))
    b_lstm = np.asarray(inputs["b_lstm"], dtype=np.float32)
    lin_W = np.asarray(inputs["lin_W"], dtype=np.float32)
    lin_b = np.asarray(inputs["lin_b"], dtype=np.float32)
    h0 = np.ascontiguousarray(np.asarray(inputs["h0"], dtype=np.float32))
    c0 = np.ascontiguousarray(np.asarray(inputs["c0"], dtype=np.float32))

    idx = np.ascontiguousarray(inp.reshape(TOK, 1).astype(np.int32))
    brep = np.ascontiguousarray(np.broadcast_to(b_lstm[None, :], (P, G4)).astype(np.float32))
    linWT = np.ascontiguousarray(lin_W.T)  # [HID, VOCAB]

    in_maps = []
    for j in range(N_CORES):
        vs = slice(j * VSH, (j + 1) * VSH)
        in_maps.append({
            "idx": idx,
            "emb": emb,
            "wih": W_ih,
            "whh": W_hh,
            "brep": brep,
            "lwt": np.ascontiguousarray(linWT[:, vs]),
            "lbrep": np.ascontiguousarray(
                np.broadcast_to(lin_b[None, vs], (P, VSH)).astype(np.float32)),
            "h0": h0,
            "c0": c0,
        })

    nc = _get_program()
    res = run_bass_kernel_spmd(nc, in_maps, core_ids=list(range(N_CORES)))

    logits = np.concatenate(
        [res.results[j]["logits"] for j in range(N_CORES)], axis=1)
    ht = res.results[0]["ht"]
    ct = res.results[0]["ct"]
    return logits, (ht, ct)
